# revision 1
# baseline (speedup 1.0000x reference)
"""DSSIM loss kernel for Trainium2 (8 NeuronCores, data-parallel over batch).

Computes (1 - mean(SSIM map)) / 2 for output/target of shape [32, 3, 512, 512],
6x6 Gaussian window (sigma=1.5), VALID padding.

Math (per channel-image):
  U  = conv(x) + conv(y) = mu1 + mu2
  D  = conv(x) - conv(y) = mu1 - mu2
  P2C = conv(x^2) + conv(y^2) + C2 = E[x^2]+E[y^2] + C2
  R2C = 2*conv(x*y) + C2 = 2*E[xy] + C2
  A = U^2/2, B = D^2/2, alpha = A - B = 2 mu1 mu2, beta = A + B = mu1^2 + mu2^2
  ssim = (alpha + C1)(R2C - alpha) / ((beta + C1)(P2C - beta))

Vertical conv on the TensorEngine as banded-matrix matmuls in fp32 (one
[128,246] stationary holding +g and -g bands; U/D/P are accumulated matmul
pairs over x, y, x^2, y^2 -- conv linearity -- so VectorE prep is just the
xy product). PSUM->SBUF copies on the ScalarEngine cast to bf16, pack the
four signals into one tile, and fold the x2 / +C2 constants into Copy's
scale/bias. Horizontal conv as bf16 shifted multiply-accumulates on the
VectorEngine (tap weights are exact fp32 immediates). SSIM formula mixes
bf16 (front) and fp32 (divide/reduce). Each core returns a [128,1]
partial-sum vector; host reduces and forms the scalar loss.
"""

import functools
import math
import time

import numpy as np

# Wall-clock of the most recent on-device SPMD execution (ns). Includes
# host<->device staging through the PJRT tunnel, so it is an upper bound on
# kernel time.
LAST_EXEC_NS = None

B, C, H, W = 32, 3, 512, 512
N_CORES = 8
IMG_PER_CORE = B // N_CORES          # 4
CHIMG = IMG_PER_CORE * C             # 12 channel-images per core
WS = 6
SIGMA = 1.5
HO = H - WS + 1                      # 507
# Vertical conv chunk starts: each chunk reads input rows [s, s+128) and
# produces output rows [s, s+123). Chunks 3/4 overlap; chunk 3 contributes
# only its first 15 rows (369..383), chunk 4 covers 384..506. All used row
# ranges start at partition 0 (engine APs require 32-aligned partition base).
CHUNK_STARTS = (0, 123, 246, 369, 384)
CHUNK_USE = (123, 123, 123, 15, 123)
N_CHUNKS = len(CHUNK_STARTS)


def _gauss_taps():
    g = np.array(
        [math.exp(-((i - WS // 2) ** 2) / (2.0 * SIGMA**2)) for i in range(WS)],
        dtype=np.float32,
    )
    g = g / g.sum()
    return [float(v) for v in g]


def _band_matrix():
    """[128, 246] fp32: columns 0:123 banded +g, columns 123:246 banded -g."""
    g = _gauss_taps()
    band = np.zeros((128, 246), dtype=np.float32)
    for m in range(123):
        for j in range(WS):
            band[m + j, m] = g[j]
            band[m + j, 123 + m] = -g[j]
    return band


@functools.lru_cache(maxsize=4)
def _build_nc(c1: float, c2: float, n_chimg: int = CHIMG, repeat: int = 1):
    import concourse.bass as bass
    import concourse.tile as tile
    from concourse import bacc, mybir

    f32 = mybir.dt.float32
    bf16 = mybir.dt.bfloat16
    Alu = mybir.AluOpType
    Act = mybir.ActivationFunctionType

    g = _gauss_taps()

    nc = bacc.Bacc("TRN2", target_bir_lowering=False, debug=False,
                   num_devices=N_CORES)
    x_dram = nc.declare_dram_parameter("x", [n_chimg, H, W], f32,
                                       isOutput=False)
    y_dram = nc.declare_dram_parameter("y", [n_chimg, H, W], f32,
                                       isOutput=False)
    band_dram = nc.declare_dram_parameter("band7", [128, 246], f32,
                                          isOutput=False)
    out_dram = nc.declare_dram_parameter("partial", [128, 1], f32,
                                         isOutput=True)

    n_cols = n_chimg * N_CHUNKS  # accumulator column per (chimg, chunk)

    with tile.TileContext(nc) as tc:
        with (
            tc.tile_pool(name="const", bufs=1) as const_pool,
            tc.tile_pool(name="inp", bufs=3) as inp_pool,
            tc.tile_pool(name="sig", bufs=2) as sig_pool,
            tc.tile_pool(name="vert", bufs=2) as vert_pool,
            tc.tile_pool(name="horiz", bufs=2) as hor_pool,
            tc.tile_pool(name="form", bufs=3) as form_pool,
            tc.tile_pool(name="psum", bufs=2,
                         space=bass.MemorySpace.PSUM) as psum_pool,
        ):
            band_sb = const_pool.tile([128, 246], f32)
            nc.sync.dma_start(band_sb[:], band_dram[:])
            band_p = band_sb[:, 0:123]
            band_n = band_sb[:, 123:246]

            acc_mat = const_pool.tile([128, n_cols], f32)
            nc.vector.memset(acc_mat[:], 0.0)

            for rep in range(repeat):
              for i in range(n_chimg):
                for ci, r0 in enumerate(CHUNK_STARTS):
                    n_rows = CHUNK_USE[ci]
                    col = i * N_CHUNKS + ci

                    xt = inp_pool.tile([128, W], f32, tag="xt")
                    nc.sync.dma_start(xt[:], x_dram[i, r0:r0 + 128, :])
                    yt = inp_pool.tile([128, W], f32, tag="yt")
                    nc.sync.dma_start(yt[:], y_dram[i, r0:r0 + 128, :])

                    # Conv is linear, so U/D/P come from accumulated matmul
                    # pairs over x, y, x^2, y^2 directly; only xy needs a
                    # VectorE product.
                    x2_t = sig_pool.tile([128, W], f32, tag="x2")
                    nc.scalar.square(x2_t[:], xt[:])
                    y2_t = sig_pool.tile([128, W], f32, tag="y2")
                    nc.scalar.square(y2_t[:], yt[:])
                    xy_t = sig_pool.tile([128, W], f32, tag="xy")
                    nc.gpsimd.tensor_mul(xy_t[:], xt[:], yt[:])

                    # Vertical conv (TensorE banded matmul, fp32); PSUM->SBUF
                    # copies cast to bf16 on ScalarE.
                    ps_u = psum_pool.tile([123, W], f32, tag="psU")
                    nc.tensor.matmul(ps_u[:], band_p, xt[:],
                                     start=True, stop=False)
                    nc.tensor.matmul(ps_u[:], band_p, yt[:],
                                     start=False, stop=True)
                    ps_d = psum_pool.tile([123, W], f32, tag="psD")
                    nc.tensor.matmul(ps_d[:], band_p, xt[:],
                                     start=True, stop=False)
                    nc.tensor.matmul(ps_d[:], band_n, yt[:],
                                     start=False, stop=True)
                    ps_p = psum_pool.tile([123, W], f32, tag="psP")
                    nc.tensor.matmul(ps_p[:], band_p, x2_t[:],
                                     start=True, stop=False)
                    nc.tensor.matmul(ps_p[:], band_p, y2_t[:],
                                     start=False, stop=True)
                    ps_r = psum_pool.tile([123, W], f32, tag="psR")
                    nc.tensor.matmul(ps_r[:], band_p, xy_t[:],
                                     start=True, stop=True)

                    # PSUM->SBUF copies on ScalarE pack the 4 signals into
                    # one [n_rows, 4, W] bf16 tile; the x0.5 and +C2 for the
                    # second-moment signals fold into Copy's scale/bias, so
                    # all horizontal tap scalars are uniform g[k].
                    v_pack = vert_pool.tile([n_rows, 4, W], bf16, tag="vpack")
                    for si, (ps, cp_scale) in enumerate(
                            ((ps_u, 1.0), (ps_d, 1.0), (ps_p, 1.0),
                             (ps_r, 2.0))):
                        if si >= 2:
                            nc.scalar.activation(
                                v_pack[:, si, :], ps[0:n_rows, :], Act.Copy,
                                bias=c2, scale=cp_scale)
                        else:
                            nc.scalar.copy(v_pack[:, si, :], ps[0:n_rows, :])

                    # One-element-shifted copy so odd taps read 4B-aligned
                    # bf16 (keeps the DVE 2x packed mode available).
                    v_odd = vert_pool.tile([n_rows, 4, W], bf16, tag="vodd")
                    nc.vector.tensor_copy(v_odd[:, :, 0:W - 1],
                                          v_pack[:, :, 1:W])

                    # Horizontal conv (VectorE bf16 shifted MACs over all 4
                    # signals at once; tap weights are exact fp32 immediates).
                    h_pack = hor_pool.tile([n_rows, 4, W], bf16, tag="hpack")
                    nc.vector.tensor_scalar(
                        h_pack[:, :, 0:HO], v_pack[:, :, 0:HO], g[0], None,
                        Alu.mult)
                    for k in range(1, WS):
                        src_t = v_pack if k % 2 == 0 else v_odd
                        k0 = k if k % 2 == 0 else k - 1
                        nc.vector.scalar_tensor_tensor(
                            h_pack[:, :, 0:HO], src_t[:, :, k0:k0 + HO], g[k],
                            h_pack[:, :, 0:HO], Alu.mult, Alu.add)

                    u_t = h_pack[:, 0, :]
                    dd_t = h_pack[:, 1, :]
                    p2c_t = h_pack[:, 2, :]
                    r2c_t = h_pack[:, 3, :]

                    # SSIM pointwise formula: bf16 front, fp32 divide/reduce.
                    a_t = form_pool.tile([n_rows, HO], bf16, tag="A")
                    nc.scalar.activation(a_t[:], u_t[0:n_rows, 0:HO],
                                         Act.Square,
                                         scale=float(1.0 / math.sqrt(2.0)))
                    b_t = form_pool.tile([n_rows, HO], bf16, tag="B")
                    nc.scalar.activation(b_t[:], dd_t[0:n_rows, 0:HO],
                                         Act.Square,
                                         scale=float(1.0 / math.sqrt(2.0)))
                    al_t = form_pool.tile([n_rows, HO], bf16, tag="al")
                    nc.vector.tensor_sub(al_t[:], a_t[:], b_t[:])
                    be_t = form_pool.tile([n_rows, HO], bf16, tag="be")
                    nc.vector.tensor_add(be_t[:], a_t[:], b_t[:])
                    n2_t = form_pool.tile([n_rows, HO], bf16, tag="n2")
                    nc.vector.tensor_sub(n2_t[:], r2c_t[0:n_rows, 0:HO],
                                         al_t[:])
                    d2f_t = form_pool.tile([n_rows, HO], bf16, tag="d2f")
                    nc.vector.tensor_sub(d2f_t[:], p2c_t[0:n_rows, 0:HO],
                                         be_t[:])
                    num_t = form_pool.tile([n_rows, HO], f32, tag="num")
                    nc.vector.scalar_tensor_tensor(
                        num_t[:], al_t[:], c1, n2_t[:], Alu.add, Alu.mult)
                    den_t = form_pool.tile([n_rows, HO], f32, tag="den")
                    nc.vector.scalar_tensor_tensor(
                        den_t[:], be_t[:], c1, d2f_t[:], Alu.add, Alu.mult)
                    rec_t = form_pool.tile([n_rows, HO], f32, tag="rec")
                    nc.vector.reciprocal_approx_fast(rec_t[:], den_t[:])
                    scr_t = form_pool.tile([n_rows, HO], f32, tag="scr")
                    nc.vector.tensor_mul(scr_t[:], num_t[:], rec_t[:])
                    nc.vector.tensor_reduce(
                        acc_mat[0:n_rows, col:col + 1], scr_t[:],
                        mybir.AxisListType.X, Alu.add)

            red = const_pool.tile([128, 1], f32)
            nc.vector.tensor_reduce(red[:], acc_mat[:], mybir.AxisListType.X,
                                    Alu.add)
            nc.sync.dma_start(out_dram[:], red[:])

    nc.compile()
    return nc


def kernel(output: np.ndarray, target: np.ndarray) -> np.ndarray:
    from concourse.bass_utils import run_bass_kernel_spmd

    x = np.ascontiguousarray(output, dtype=np.float32)
    y = np.ascontiguousarray(target, dtype=np.float32)
    assert x.shape == (B, C, H, W) and y.shape == (B, C, H, W)

    mx = float(x.max())
    mn = float(x.min())
    max_val = 255.0 if mx > 128.0 else 1.0
    min_val = -1.0 if mn < -0.5 else 0.0
    L = max_val - min_val
    c1 = float((0.01 * L) ** 2)
    c2 = float((0.03 * L) ** 2)

    nc = _build_nc(c1, c2)

    band = _band_matrix()
    in_maps = []
    for core in range(N_CORES):
        sl = slice(core * IMG_PER_CORE, (core + 1) * IMG_PER_CORE)
        in_maps.append({
            "x": np.ascontiguousarray(x[sl].reshape(CHIMG, H, W)),
            "y": np.ascontiguousarray(y[sl].reshape(CHIMG, H, W)),
            "band7": band,
        })

    global LAST_EXEC_NS
    t0 = time.perf_counter()
    res = run_bass_kernel_spmd(nc, in_maps, list(range(N_CORES)))
    LAST_EXEC_NS = int((time.perf_counter() - t0) * 1e9)
    total = 0.0
    for core in range(N_CORES):
        total += float(res.results[core]["partial"].astype(np.float64).sum())
    mean_ssim = total / float(B * C * HO * HO)
    return np.asarray((1.0 - mean_ssim) / 2.0, dtype=np.float32)

